# revision 26
# baseline (speedup 1.0000x reference)
"""BiCutLoss Trainium2 kernel (8-core data parallel over batch).

Reference semantics (B=16384, L=1024):
    temp[b,j]  = argmax(output[b,j,:])          # 1 iff out1 > out0 (ties -> 0)
    idx[b]     = L if row all-ones else index of last zero
    mask[b,j]  = j <= idx[b]
    r1[b,j]    = -1/log2(j+2)  if labels==1 else (j+1)/alpha
    loss       = sum(output[...,1] * mask * r1) / B

Restructuring: masked_sum = full_sum - tail_sum. The tail (j > idx) is
confined to the last W columns whenever each row has a zero decision
there (P(violation) = 2^-W per row for +-symmetric data; a per-core
flag count detects it and the host falls back to exact numpy, so the
kernel is correct for all inputs).

fp16 pipeline (loss rel-err ~7e-5, budget 2e-2): out1 as f16 (4 MB/core),
labels as u8 (2 MB/core) upcast on ScalarE/DVE, window out0 as packed
f16. Work is chunk-granular (4 tiles = [128, 4096]) to amortize
per-instruction overhead. The four per-tile column-sum matmuls
(out1 lo/hi, ql lo/hi; all M=1) run CONCURRENTLY via PE column tiling:
tile_position (0, 32g) with outputs on psum partitions 0/32/64/96 of
one bank. Epilogue: copy psum bank -> SBUF, weighted elementwise dot
against a host-built [97, 512] coefficient sheet (zero rows except the
four group rows), per-partition accum, and a tiny matmul contracts the
partial dots across partitions. Tail/flag strip accumulates in a second
psum bank (single chain; a chain's start clears its whole bank).
"""

import threading
from contextlib import ExitStack

import numpy as np

B, L = 16384, 1024
N_CORES = 8
ROWS_PER_CORE = B // N_CORES  # 2048
ALPHA = 0.65
W = 32              # tail window width
N_TILES = 16        # [128, 1024] tiles per core
CH = 4              # tiles per DMA chunk
N_CHUNKS = N_TILES // CH
WSTRIP = 3 * W * CH  # strip: CH x [tq|tl|s], accumulated over chunks
DVE_CAST_CHUNKS = ()  # chunks whose u8->f16 cast runs on DVE (rest ScalarE)

_compiled = threading.local()


def _reward_rows():
    j = np.arange(L, dtype=np.float64)
    bv = (j + 1.0) / ALPHA
    d = -1.0 / np.log2(j + 2.0) - bv
    return bv, d


def _coeffs():
    bv, d = _reward_rows()
    crow97 = np.zeros((128, 512), dtype=np.float32)
    crow97[0] = bv[0:512]
    crow97[32] = bv[512:L]
    crow97[64] = d[0:512]
    crow97[96] = d[512:L]
    cstrip = np.concatenate(
        [np.concatenate([-bv[L - W :], -d[L - W :], np.zeros(W)]) for _ in range(CH)]
    ).astype(np.float32).reshape(1, WSTRIP)
    w97 = np.zeros((97, 1), dtype=np.float32)
    w97[[0, 32, 64, 96]] = 1.0
    return crow97, cstrip, w97


def _build(rows=ROWS_PER_CORE, num_devices=N_CORES, dump=False):
    import concourse.tile as tile
    from concourse import bacc, mybir

    f32 = mybir.dt.float32
    f16 = mybir.dt.float16
    u8 = mybir.dt.uint8
    Alu = mybir.AluOpType
    Act = mybir.ActivationFunctionType

    nc = bacc.Bacc(
        "TRN2",
        target_bir_lowering=False,
        debug=False,
        enable_asserts=True,
        num_devices=num_devices,
    )

    out1_d = nc.dram_tensor("out1", [rows, L], f16, kind="ExternalInput").ap()
    lab_d = nc.dram_tensor("lab", [rows, L], u8, kind="ExternalInput").ap()
    w0_d = nc.dram_tensor("w0", [128, N_TILES * W], f16, kind="ExternalInput").ap()
    crow_d = nc.dram_tensor("crow97", [128, 512], f32, kind="ExternalInput").ap()
    cstrip_d = nc.dram_tensor("cstrip", [1, WSTRIP], f32, kind="ExternalInput").ap()
    res_d = nc.dram_tensor("partial", [1, 8], f32, kind="ExternalOutput").ap()
    if dump:
        dump_d = nc.dram_tensor("dump", [97, 512 + WSTRIP], f32, kind="ExternalOutput").ap()

    rows_per_chunk = rows // N_CHUNKS  # 512
    CL = CH * L                        # chunk columns (4096)

    with tile.TileContext(nc) as tc, ExitStack() as ctx:
        const = ctx.enter_context(tc.tile_pool(name="const", bufs=1))
        o1p = ctx.enter_context(tc.tile_pool(name="o1p", bufs=N_CHUNKS))
        lbp = ctx.enter_context(tc.tile_pool(name="lbp", bufs=N_CHUNKS))
        lfp = ctx.enter_context(tc.tile_pool(name="lfp", bufs=2))
        qlp = ctx.enter_context(tc.tile_pool(name="qlp", bufs=2))
        wk = ctx.enter_context(tc.tile_pool(name="wk", bufs=2))
        psum = ctx.enter_context(tc.tile_pool(name="psum", bufs=1, space="PSUM"))

        ones = const.tile([128, 1], f16)
        nc.vector.memset(ones[:], 1.0)
        # w97 built by memsets: a [97, 1] DMA is 97 tiny descriptors
        w97t = const.tile([97, 1], f32)
        nc.vector.memset(w97t[:], 0.0)
        for g in range(4):
            nc.vector.memset(w97t[32 * g : 32 * g + 1, 0:1], 1.0)

        # psum: bank 0 = 4 column-sum groups on partitions 0/32/64/96;
        # bank 1 = tail/flag strip (partition 0). ps2 = final cross-
        # partition dot (separate region; values written by DVE/copy are
        # never cleared by other chains - clear only resets has_written).
        ps = psum.tile([97, 512 + WSTRIP], f32)
        ps2 = psum.tile([1, 1], f32)
        # zero bank 0 so unused partitions read 0.0 (not NaN garbage)
        nc.vector.memset(ps[0:97, 0:512], 0.0)

        # all input DMAs on the sync HWDGE ring, in consumption order;
        # epilogue-only constants issue last so they never delay chunk 0
        w0t = const.tile([128, N_TILES * W], f16)
        crow97 = const.tile([128, 512], f32)
        cstrip = const.tile([1, WSTRIP], f32)
        # labels ride the scalar HWDGE ring, out1 the sync ring: two
        # independent FIFOs issue and drain in parallel. Chunk 0 is
        # split in half so its cast can start ~2 rows-quarters earlier.
        chunks = []
        for c in range(N_CHUNKS):
            r0 = c * rows_per_chunk
            lbc = lbp.tile([128, CL], u8, tag="lb")
            o1c = o1p.tile([128, CL], f16, tag="o1")
            halves = 2 if c == 0 else 1
            hr = rows_per_chunk // halves
            hq = CH // halves
            for h in range(halves):
                nc.scalar.dma_start(
                    lbc[:, h * (CL // halves) : (h + 1) * (CL // halves)]
                    .rearrange("p (q l) -> p q l", q=hq),
                    lab_d[r0 + h * hr : r0 + (h + 1) * hr, :].rearrange(
                        "(p q) l -> p q l", q=hq
                    ),
                )
                nc.sync.dma_start(
                    o1c[:, h * (CL // halves) : (h + 1) * (CL // halves)]
                    .rearrange("p (q l) -> p q l", q=hq),
                    out1_d[r0 + h * hr : r0 + (h + 1) * hr, :].rearrange(
                        "(p q) l -> p q l", q=hq
                    ),
                )
            chunks.append((o1c, lbc))
            if c == 0:
                nc.sync.dma_start(w0t[:], w0_d[:])
        nc.sync.dma_start(crow97[:], crow_d[:])
        nc.sync.dma_start(cstrip[:], cstrip_d[:])

        for c in range(N_CHUNKS):
            o1c, lbc = chunks[c]
            st, sp = c == 0, c == N_CHUNKS - 1

            # labels u8 -> f16
            lf = lfp.tile([128, CL], f16, tag="lf")
            if c in DVE_CAST_CHUNKS:
                nc.vector.tensor_copy(lf[:], lbc[:])
            else:
                nc.scalar.activation(lf[:], lbc[:], Act.Copy)

            # ql = out1 * lab (DVE fp16 2x)
            ql = qlp.tile([128, CL], f16, tag="ql")
            nc.vector.tensor_tensor(ql[:], o1c[:], lf[:], Alu.mult)

            # --- batched window pipeline over the chunk's CH tiles ---
            def wview(base_ap):
                return base_ap.rearrange("p (q l) -> p q l", q=CH)[:, :, L - W : L]

            w1v = wview(o1c[:])
            lwv = wview(lf[:])
            w0v = w0t[:, c * CH * W : (c + 1) * CH * W].rearrange(
                "p (q w) -> p q w", q=CH
            )

            w12 = wk.tile([128, WSTRIP], f16, tag="w12")
            w12v = w12[:].rearrange("p (q x) -> p q x", q=CH)
            tqv = w12v[:, :, 0:W]
            tlv = w12v[:, :, W : 2 * W]
            sv_ = w12v[:, :, 2 * W : 3 * W]

            ge = wk.tile([128, CH * W], f16, tag="ge")
            gev = ge[:].rearrange("p (q w) -> p q w", q=CH)
            nc.vector.tensor_tensor(gev, w0v, w1v, Alu.is_ge)
            for q in range(CH):  # suffix-max per tile (scan can't batch)
                s_q = w12[:, 3 * W * q + 2 * W : 3 * W * (q + 1)]
                g_q = ge[:, W * q : W * (q + 1)]
                nc.vector.tensor_tensor_scan(
                    s_q[:, ::-1], g_q[:, ::-1], g_q[:, ::-1],
                    0.0, Alu.max, Alu.max,
                )
            # tm = s0 - s (stride-0 broadcast of each tile's s column 0)
            tm = wk.tile([128, CH * W], f16, tag="tm")
            tmv = tm[:].rearrange("p (q w) -> p q w", q=CH)
            s0b = w12v[:, :, 2 * W : 2 * W + 1].broadcast_to([128, CH, W])
            nc.vector.tensor_tensor(tmv, s0b, sv_, Alu.subtract)
            nc.vector.tensor_tensor(tqv, tmv, w1v, Alu.mult)
            nc.vector.tensor_tensor(tlv, tqv, lwv, Alu.mult)

            # column sums: 4 concurrent PE column groups per tile
            for q in range(CH):
                o1 = o1c[:, q * L : (q + 1) * L]
                qlt = ql[:, q * L : (q + 1) * L]
                qst = st and q == 0
                qsp = sp and q == CH - 1
                nc.tensor.matmul(ps[0:1, 0:512], ones[:], o1[:, 0:512], start=qst, stop=qsp, tile_position=(0, 0))
                nc.tensor.matmul(ps[32:33, 0:512], ones[:], o1[:, 512:L], start=qst, stop=qsp, tile_position=(0, 32))
                nc.tensor.matmul(ps[64:65, 0:512], ones[:], qlt[:, 0:512], start=qst, stop=qsp, tile_position=(0, 64))
                nc.tensor.matmul(ps[96:97, 0:512], ones[:], qlt[:, 512:L], start=qst, stop=qsp, tile_position=(0, 96))
            nc.tensor.matmul(
                ps[0:1, 512 : 512 + WSTRIP], ones[:], w12[:], start=st, stop=sp
            )

        # ---- epilogue ----
        sv = const.tile([97, 512], f32)
        nc.scalar.copy(sv[:], ps[0:97, 0:512])
        junk97 = const.tile([97, 512], f32)
        acc97 = const.tile([97, 1], f32)
        nc.vector.scalar_tensor_tensor(
            junk97[:], sv[:], 1.0, crow97[0:97, :], Alu.mult, Alu.mult,
            accum_out=acc97[:],
        )
        res = const.tile([1, 8], f32)
        junkS = const.tile([1, WSTRIP], f32)
        nc.vector.scalar_tensor_tensor(
            junkS[:], ps[0:1, 512 : 512 + WSTRIP], 1.0, cstrip[:],
            Alu.mult, Alu.mult, accum_out=res[0:1, 1:2],
        )
        nc.tensor.matmul(ps2[0:1, 0:1], w97t[:], acc97[:], start=True, stop=True)
        nc.scalar.copy(res[0:1, 0:1], ps2[0:1, 0:1])
        # flag counts: s column 0 of each strip sub-block
        nc.scalar.activation(
            res[0:1, 2 : 2 + CH],
            ps[0:1, 512 + 2 * W : 512 + WSTRIP : 3 * W],
            Act.Copy,
        )
        nc.vector.memset(res[0:1, 6:8], 0.0)
        nc.sync.dma_start(res_d[:], res[:])
        if dump:
            psc = const.tile([97, 512 + WSTRIP], f32)
            nc.scalar.copy(psc[:], ps[:])
            nc.scalar.dma_start(dump_d[:], psc[:])

    nc.compile()
    return nc


def _get_nc():
    if getattr(_compiled, "nc", None) is None:
        _compiled.nc = _build()
    return _compiled.nc


def _in_maps(output, labels):
    out1 = output[:, :, 1].astype(np.float16)
    w0 = output[:, L - W :, 0].astype(np.float16)  # [B, W]
    lab = labels.astype(np.uint8)
    crow97, cstrip, w97 = _coeffs()
    rp = ROWS_PER_CORE
    maps = []
    for c in range(N_CORES):
        w0c = w0[c * rp : (c + 1) * rp]  # [2048, W]
        # chunks 1-3: tile (ch,q) <- DRAM row ch*512 + p*4 + q.
        # chunk 0 is DMA'd as two halves: block h*2+q <- row 256h + 2p + q.
        w0pack = (
            w0c.reshape(N_CHUNKS, 128, CH, W).transpose(1, 0, 2, 3).copy()
        )  # [128, ch, q, W]
        w0pack[:, 0] = w0c[0:512].reshape(2, 128, 2, W).transpose(1, 0, 2, 3).reshape(128, CH, W)
        w0pack = np.ascontiguousarray(w0pack).reshape(128, N_TILES * W)
        maps.append(
            {
                "out1": np.ascontiguousarray(out1[c * rp : (c + 1) * rp]),
                "lab": np.ascontiguousarray(lab[c * rp : (c + 1) * rp]),
                "w0": w0pack,
                "crow97": crow97,
                "cstrip": cstrip,
            }
        )
    return maps


def _host_fallback(output, labels):
    temp = output[:, :, 1] > output[:, :, 0]
    allones = temp.all(axis=1)
    z = ~temp
    last_zero = (L - 1) - np.argmax(z[:, ::-1], axis=1)
    idx = np.where(allones, L, last_zero)
    mask = np.arange(L)[None, :] <= idx[:, None]
    j = np.arange(L, dtype=np.float64)
    r1 = np.where(labels == 1, -1.0 / np.log2(j + 2.0), (j + 1.0) / ALPHA)
    return np.float32(
        (output[:, :, 1].astype(np.float64) * mask * r1).sum() / B
    )


def _combine(results, output, labels):
    total = 0.0
    flags = 0.0
    for r in results:
        p = np.asarray(r["partial"], dtype=np.float64)
        total += p[0, 0] + p[0, 1]
        flags += p[0, 2 : 2 + CH].sum()
    if flags != B:
        # some row has no zero decision in its last-W window: either a
        # genuine all-ones row (kernel already correct: tail = 0) or a row
        # whose last zero is before the window (kernel overcounts). The
        # f16-exact check below distinguishes; fall back only when needed.
        # Never fires for +-symmetric random inputs (P ~ B * 2^-W).
        o0 = output[:, L - W :, 0].astype(np.float16)
        o1 = output[:, L - W :, 1].astype(np.float16)
        haszero = (o0 >= o1).any(axis=1)
        allones_f16 = ~(
            (output[:, :, 0].astype(np.float16) >= output[:, :, 1].astype(np.float16))
        ).any(axis=1)
        if (~haszero & ~allones_f16).any():
            return _host_fallback(output, labels)
    return np.float32(total / B)


def kernel(output: np.ndarray, labels: np.ndarray) -> np.ndarray:
    from concourse.bass_utils import run_bass_kernel_spmd

    assert output.shape == (B, L, 2), output.shape
    nc = _get_nc()
    res = run_bass_kernel_spmd(
        nc, _in_maps(output, labels), core_ids=list(range(N_CORES))
    )
    return _combine(res.results, output, labels)
